# revision 2
# baseline (speedup 1.0000x reference)
"""Trainium2 Bass kernel v2 for ContrastHead (softnn contrastive KNN loss).

Restructure vs v1: (1) signed int16 gather indices with mid-window bases cover
65536 rows per window -> 2 windows instead of 4; (2) host drops the ~15% of
points excluded by point_mask; (3) points sorted by their window-0 neighbor
count c0 so each supertile's window-0 gather covers a static column prefix
[0, CA) and window-1 a static prefix [0, CB) of a disjoint column range --
descgen work ~ (c0max + c1max + margins) cols instead of 2*31; trailing
all-dummy margin columns carry idx -32768 which the Q7 ucode trims at runtime.
Dummy slots gather Z rows with norm 1e9 (=> exp contribution 0); gather
buffers are pre-memset to f16 2048.0 so slots skipped by the trim are equally
inert. Per-point reductions stay canonical: each point's real slots live in
its own partition row inside [0, CA)+[CA, CA+CB). No OR-merge needed.
"""

import numpy as np

import concourse.bacc as bacc
import concourse.bass as bass
import concourse.mybir as mybir
import concourse.tile as tile
from concourse import bass_utils

F16 = mybir.dt.float16
F32 = mybir.dt.float32
I16 = mybir.dt.int16
I32 = mybir.dt.int32

N = 100000
K = 31
C = 64
ROW = 128                    # f16 slots per table row (256B)
NCORES = 8

W0MAX = 65534                # data rows 0..W0MAX belong to window 0
PHYS = 100002                # [Z0][data 0..65534][Z1][data 65535..99999]
B0 = 32768                   # window-0 base phys row (idx -32768..32767 -> 0..65535)
B1 = 98304                   # window-1 base phys row (idx -32768 -> Z1 at 65536)
Q0 = (W0MAX + 1) / N         # P(neighbor in window 0)

TPC = 83                     # tiles (of 128 points) per core; 83*128=10624 >= ~10593
G = 4                        # tiles per supertile
NST = (TPC + G - 1) // G     # 21 supertiles (20x4 + 1x3)
TEMP = 0.1
EPS = 1e-8
FILL = 2048.0                # f16 memset fill: f32-reinterp ~2.4e24 => exp -> 0

_CACHE = {}


def _binom_ppf_table():
    """CDF of Binomial(K, Q0) for the static column bounds."""
    from math import comb

    pmf = np.array([comb(K, i) * Q0**i * (1 - Q0) ** (K - i) for i in range(K + 1)])
    return np.cumsum(pmf)


def _static_bounds():
    """Per-supertile static (CA, CB) column counts, data-independent."""
    cdf = _binom_ppf_table()
    npts = TPC * 128
    eps_q = 0.05
    bounds = []
    for s in range(NST):
        g_s = min(G, TPC - s * G)
        hi = min((s * G + g_s) * 128 / npts + eps_q, 1.0)
        lo = max(s * G * 128 / npts - eps_q, 0.0)
        ca = min(K, int(np.searchsorted(cdf, hi)) + 3)
        c0min = max(0, int(np.searchsorted(cdf, lo)) - 3)
        cb = K - c0min
        bounds.append((g_s, ca, cb))
    return bounds

BOUNDS = _static_bounds()


def _build():
    nc = bacc.Bacc("TRN2", target_bir_lowering=False, debug=False)

    tabT = nc.dram_tensor("tab", (PHYS, ROW), F16, kind="ExternalInput")
    selfT = nc.dram_tensor("selftab", (128, TPC, ROW), F16, kind="ExternalInput")
    validT = nc.dram_tensor("valid", (128, TPC), F32, kind="ExternalInput")
    nibA = sum(ca * g_s * 8 for g_s, ca, _ in BOUNDS)
    nibB = sum(cb * g_s * 8 for g_s, _, cb in BOUNDS)
    idxAT = nc.dram_tensor("idxA", (128, nibA), I16, kind="ExternalInput")
    idxBT = nc.dram_tensor("idxB", (128, nibB), I16, kind="ExternalInput")
    cntT = nc.dram_tensor("cnts", (1, 2 * NST), I32, kind="ExternalInput")
    outT = nc.dram_tensor("out", (128, 2), F32, kind="ExternalOutput")

    with tile.TileContext(nc) as tc:
        with (
            tc.tile_pool(name="res", bufs=1) as res,
            tc.tile_pool(name="gpool", bufs=2) as gpool,
            tc.tile_pool(name="ipool", bufs=2) as ipool,
            tc.tile_pool(name="mpool", bufs=1) as mpool,
            tc.tile_pool(name="dpool", bufs=2) as dpool,
            tc.tile_pool(name="p2", bufs=1) as p2,
        ):
            selfsb = res.tile([128, TPC, ROW], F16)
            nc.sync.dma_start(out=selfsb[:], in_=selfT.ap())
            validsb = res.tile([128, TPC], F32)
            nc.sync.dma_start(out=validsb[:], in_=validT.ap())

            cntsb = res.tile([1, 2 * NST], I32)
            nc.sync.dma_start(out=cntsb[:], in_=cntT.ap())
            regA = nc.alloc_register(mybir.EngineType.Pool, "gcntA")
            regB = nc.alloc_register(mybir.EngineType.Pool, "gcntB")

            negT = res.tile([128, TPC], F32)
            posT = res.tile([128, TPC], F32)
            selff32 = selfsb[:].bitcast(F32)          # (128, TPC, 64)

            offA = 0
            offB = 0
            for s in range(NST):
                g_s, CA, CB = BOUNDS[s]
                t0 = s * G
                CAB = CA + CB
                LA = CA * g_s * 128
                LB = CB * g_s * 128

                ia = ipool.tile([128, LA // 16], I16, tag="ia")
                nc.sync.dma_start(out=ia[:], in_=idxAT.ap()[:, offA : offA + LA // 16])
                ib = ipool.tile([128, LB // 16], I16, tag="ib")
                nc.sync.dma_start(out=ib[:], in_=idxBT.ap()[:, offB : offB + LB // 16])
                offA += LA // 16
                offB += LB // 16

                gA = gpool.tile([128, CA, g_s, ROW], F16, tag="gA")
                nc.vector.memset(gA[:], FILL)
                nc.reg_load(regA, cntsb[:, 2 * s : 2 * s + 1])
                nc.gpsimd.dma_gather(
                    out_ap=gA[:].rearrange("p a g r -> p (a g) r"),
                    in_ap=tabT.ap()[B0 : B0 + 32768, :],
                    idxs_ap=ia[:],
                    num_idxs=LA,
                    num_idxs_reg=regA,
                    elem_size=ROW,
                    single_packet=False,
                )
                gB = gpool.tile([128, CB, g_s, ROW], F16, tag="gB")
                nc.vector.memset(gB[:], FILL)
                nc.reg_load(regB, cntsb[:, 2 * s + 1 : 2 * s + 2])
                nc.gpsimd.dma_gather(
                    out_ap=gB[:].rearrange("p a g r -> p (a g) r"),
                    in_ap=tabT.ap()[B1:PHYS, :],
                    idxs_ap=ib[:],
                    num_idxs=LB,
                    num_idxs_reg=regB,
                    elem_size=ROW,
                    single_packet=False,
                )

                d2 = dpool.tile([128, g_s, CAB], F32, tag="d2")
                pm = dpool.tile([128, g_s, CAB], F32, tag="pm")
                gAf32 = gA[:].bitcast(F32)            # (128, CA, g_s, 64)
                gBf32 = gB[:].bitcast(F32)

                for g in range(g_s):
                    t = t0 + g
                    for (gt, gf32, CW, c_off) in (
                        (gA, gAf32, CA, 0),
                        (gB, gBf32, CB, CA),
                    ):
                        m = mpool.tile([128, CW, C], F16, tag=f"m{c_off == 0}")
                        fb = selfsb[:, t, 0:C].unsqueeze(1).broadcast_to([128, CW, C])
                        nc.vector.tensor_tensor(
                            out=m[:], in0=gt[:, :, g, 0:C], in1=fb,
                            op=mybir.AluOpType.mult,
                        )
                        nc.vector.tensor_add(
                            out=m[:, :, 0:32], in0=m[:, :, 0:32], in1=m[:, :, 32:64]
                        )
                        nc.vector.tensor_add(
                            out=m[:, :, 0:16], in0=m[:, :, 0:16], in1=m[:, :, 16:32]
                        )
                        nc.vector.tensor_add(
                            out=m[:, :, 0:8], in0=m[:, :, 0:8], in1=m[:, :, 8:16]
                        )
                        dsl = d2[:, g, c_off : c_off + CW]
                        nc.vector.reduce_sum(
                            out=dsl, in_=m[:, :, 0:8], axis=mybir.AxisListType.X
                        )
                        sj = gf32[:, :, g, 32]           # (128, CW)
                        nc.vector.scalar_tensor_tensor(
                            out=dsl, in0=dsl, scalar=-2.0, in1=sj,
                            op0=mybir.AluOpType.mult, op1=mybir.AluOpType.add,
                        )
                        si = selff32[:, t, 32].unsqueeze(1).broadcast_to([128, CW])
                        nc.vector.tensor_add(out=dsl, in0=dsl, in1=si)
                        nl = gt[:, :, g, 66]             # (128, CW) labels
                        li = selfsb[:, t, 66].unsqueeze(1).broadcast_to([128, CW])
                        nc.vector.tensor_tensor(
                            out=pm[:, g, c_off : c_off + CW], in0=nl, in1=li,
                            op=mybir.AluOpType.is_equal,
                        )

                # phase 2 per supertile
                nc.scalar.sqrt(out=d2[:], in_=d2[:])
                mind = dpool.tile([128, g_s], F32, tag="mind")
                nc.vector.tensor_reduce(
                    out=mind[:], in_=d2[:], axis=mybir.AxisListType.X,
                    op=mybir.AluOpType.min,
                )
                mbc = mind[:].unsqueeze(2).broadcast_to([128, g_s, CAB])
                nc.vector.tensor_tensor(
                    out=d2[:], in0=d2[:], in1=mbc, op=mybir.AluOpType.subtract
                )
                nc.scalar.activation(
                    out=d2[:], in_=d2[:],
                    func=mybir.ActivationFunctionType.Exp, scale=-1.0 / TEMP,
                )
                nc.vector.reduce_sum(
                    out=negT[:, t0 : t0 + g_s], in_=d2[:], axis=mybir.AxisListType.X
                )
                nc.vector.tensor_tensor(
                    out=d2[:], in0=d2[:], in1=pm[:], op=mybir.AluOpType.mult
                )
                nc.vector.reduce_sum(
                    out=posT[:, t0 : t0 + g_s], in_=d2[:], axis=mybir.AxisListType.X
                )

            # final
            rn = p2.tile([128, TPC], F32)
            nc.vector.reciprocal(out=rn[:], in_=negT[:])
            ratio = p2.tile([128, TPC], F32)
            nc.vector.tensor_tensor(
                out=ratio[:], in0=posT[:], in1=rn[:], op=mybir.AluOpType.mult
            )
            eps_t = p2.tile([128, 1], F32)
            nc.vector.memset(eps_t[:], EPS)
            lg = p2.tile([128, TPC], F32)
            nc.scalar.activation(
                out=lg[:], in_=ratio[:],
                func=mybir.ActivationFunctionType.Ln, bias=eps_t[:],
            )
            nc.vector.tensor_tensor(
                out=lg[:], in0=lg[:], in1=validsb[:], op=mybir.AluOpType.mult
            )
            outsb = p2.tile([128, 2], F32)
            nc.vector.reduce_sum(out=outsb[:, 0:1], in_=lg[:], axis=mybir.AxisListType.X)
            nc.vector.reduce_sum(out=outsb[:, 1:2], in_=validsb[:], axis=mybir.AxisListType.X)
            nc.sync.dma_start(out=outT.ap(), in_=outsb[:])

    nc.compile()
    return nc


def _get_nc():
    if "nc" not in _CACHE:
        _CACHE["nc"] = _build()
    return _CACHE["nc"]


def _phys(r):
    return r + 1 + (r >= W0MAX + 1)


def _pack_table(features, labels):
    tab = np.zeros((PHYS, ROW), dtype=np.float16)
    pr = _phys(np.arange(N))
    tab[pr, 0:C] = features.astype(np.float16)
    s = np.sum(features.astype(np.float64) ** 2, axis=1).astype(np.float32)
    tab[pr, 64:66] = s[:, None].view(np.float16)
    tab[pr, 66] = labels.astype(np.float16)
    # Z rows: features 0, norm 1e9 (exp -> 0), label -1 (never matches)
    for z in (0, W0MAX + 2):
        tab[z, 64:66] = np.array([1e9], dtype=np.float32).view(np.float16)
        tab[z, 66] = -1.0
    return tab


def _wrap16(flat):
    nib = flat.shape[0] // 16
    w = flat.reshape(nib, 16).T.astype(np.int16)
    return np.tile(w, (8, 1))


def _prep(features, labels, neighbor_idx):
    tab = _pack_table(features, labels)
    nl = labels[neighbor_idx]
    pmk = labels[:, None] == nl
    cnt = pmk.sum(1)
    keep = np.nonzero((cnt > 0) & (cnt < K))[0]

    inw0 = neighbor_idx[keep] <= W0MAX                      # (M, K)
    c0 = inw0.sum(1).astype(np.int64)
    order = np.argsort(c0, kind="stable")
    pts_sorted = keep[order]
    c0s = c0[order]

    # w0-neighbors first within each row (original order preserved)
    perm = np.argsort(~inw0[order], axis=1, kind="stable")
    sn = np.take_along_axis(neighbor_idx[keep][order], perm, axis=1)  # (M, K)

    ar = np.arange(K)[None, :]
    a_idx = np.where(ar < c0s[:, None], _phys(sn) - B0, -32768).astype(np.int16)
    b_src = np.take_along_axis(sn, np.minimum(c0s[:, None] + ar, K - 1), axis=1)
    b_idx = np.where(
        c0s[:, None] + ar < K, _phys(b_src) - B1, -32768
    ).astype(np.int16)

    in_maps = []
    npts = TPC * 128
    for core in range(NCORES):
        sel = np.arange(core, len(pts_sorted), NCORES)
        m = len(sel)
        pts_c = pts_sorted[sel]
        ac = np.full((npts, K), -32768, dtype=np.int16)
        bc = np.full((npts, K), -32768, dtype=np.int16)
        ac[:m] = a_idx[sel]
        bc[:m] = b_idx[sel]
        c0c = np.zeros(npts, dtype=np.int64)
        c0c[:m] = c0s[sel]

        selfpack = np.zeros((npts, ROW), dtype=np.float16)
        selfpack[:m] = tab[_phys(pts_c)]
        selfpack[m:, 66] = -2.0
        valid = np.zeros(npts, dtype=np.float32)
        valid[:m] = 1.0

        segA = []
        segB = []
        cnts = np.zeros(2 * NST, dtype=np.int32)
        for s in range(NST):
            g_s, CA, CB = BOUNDS[s]
            lo = s * G * 128
            hi = lo + g_s * 128
            if c0c[lo:min(hi, m)].size:
                assert c0c[lo:min(hi, m)].max(initial=0) <= CA, (
                    f"supertile {s}: c0 max {c0c[lo:hi].max()} > CA {CA}"
                )
                assert (K - c0c[lo:min(hi, m)].min(initial=K)) <= CB, (
                    f"supertile {s}: c1 max > CB {CB}"
                )
            # (g_s*128, CA) -> (g_s, 128, CA) -> (CA, g_s, 128) -> flat
            sa = ac[lo:hi, :CA].reshape(g_s, 128, CA).transpose(2, 0, 1).reshape(-1)
            sb = bc[lo:hi, :CB].reshape(g_s, 128, CB).transpose(2, 0, 1).reshape(-1)
            segA.append(sa)
            segB.append(sb)
            nzA = np.nonzero(sa >= 0)[0]
            nzB = np.nonzero(sb >= 0)[0]
            cnts[2 * s] = (nzA[-1] + 1) if nzA.size else 0
            cnts[2 * s + 1] = (nzB[-1] + 1) if nzB.size else 0

        in_maps.append({
            "tab": tab,
            "selftab": np.ascontiguousarray(
                selfpack.reshape(TPC, 128, ROW).transpose(1, 0, 2)
            ),
            "valid": np.ascontiguousarray(
                valid.reshape(TPC, 128).transpose(1, 0)
            ).astype(np.float32),
            "idxA": np.ascontiguousarray(np.concatenate([_wrap16(x) for x in segA], axis=1)),
            "idxB": np.ascontiguousarray(np.concatenate([_wrap16(x) for x in segB], axis=1)),
            "cnts": cnts[None, :],
        })
    return in_maps


def run(features, labels, neighbor_idx, trace=False):
    nc = _get_nc()
    in_maps = _prep(
        np.asarray(features), np.asarray(labels), np.asarray(neighbor_idx)
    )
    res = bass_utils.run_bass_kernel_spmd(
        nc, in_maps, core_ids=list(range(NCORES)), trace=trace
    )
    s = 0.0
    ccnt = 0.0
    for o in res.results:
        s += float(o["out"][:, 0].astype(np.float64).sum())
        ccnt += float(o["out"][:, 1].astype(np.float64).sum())
    loss = np.float32(-s / max(ccnt, 1.0))
    return loss, res


def kernel(features, labels, neighbor_idx):
    loss, _ = run(features, labels, neighbor_idx, trace=False)
    return loss
